# revision 1
# baseline (speedup 1.0000x reference)
"""DisentangledSelfAttention (DeBERTa-style) Trainium2 Bass kernel.

Sharding: 8 cores = 4 batches x 2 head-halves. Each core computes one batch
with 8 heads; the output projection is row-parallel, partial outputs are
summed pairwise on the host (unshard step).

Core algorithm notes:
 - scores are assembled TRANSPOSED (scoresT[j, s]) so softmax normalization
   folds into the ctx matmul via an appended ones-column (row 64 of ctxT
   psum = denominator) and probsT feeds the ctx matmul with no transpose.
 - the rel-position gathers (c2p/p2c) become diagonal "band" reads of
   expanded tables. Expansion happens on the PE via one-hot matmuls with
   host-precomputed Msel matrices; tables round-trip through DRAM so the
   diagonal read is a flat (affine) DRAM access pattern. The expansion axis
   is reversed (d = 1023 - E) so band reads are contiguous.
"""

import os
import sys

import numpy as np

for _p in ("/opt/trn_rl_repo", "/root/.axon_site/_ro/trn_rl_repo"):
    if os.path.isdir(_p) and _p not in sys.path:
        sys.path.insert(0, _p)

import ml_dtypes  # noqa: E402
import concourse.bass as bass  # noqa: E402
import concourse.bacc as bacc  # noqa: E402
import concourse.mybir as mybir  # noqa: E402
import concourse.tile as tile  # noqa: E402
import concourse.masks as masks  # noqa: E402
from concourse.bass_utils import run_bass_kernel_spmd  # noqa: E402

F32 = mybir.dt.float32
F32R = mybir.dt.float32r
BF16 = mybir.dt.bfloat16
AF = mybir.ActivationFunctionType
ALU = mybir.AluOpType
AX = mybir.AxisListType
BF16NP = ml_dtypes.bfloat16

B, S, D, H, HD = 4, 1024, 1024, 16, 64
NCORES = 8
NH = 8          # heads per core
CH = 512        # feature columns per core
SPAN, K2 = 256, 512
EW = 2048       # expanded table width (global E axis; d = 1023 - E)
TW = 1152       # per-block table window width
NB = 8          # 128-row blocks in S
P = 128

SC_Q = float(1.0 / np.sqrt(HD * 3))     # folded into qT
SC_PK = float(np.sqrt(1.5))             # c2p table extra scale (qT carries 1/sqrt(192))
SC_PQ = float(1.0 / np.sqrt(HD * 2))    # p2c table scale
CHUNKS = ((0, 512), (512, 512), (1024, 128))  # TW split into psum-bank matmuls


def _np_log_bucket(rel, bucket_size=SPAN, max_position=512):
    rel = np.asarray(rel)
    mid = bucket_size // 2
    sign = np.sign(rel).astype(np.float32)
    abs_pos = np.where((rel < mid) & (rel > -mid), np.float32(mid - 1),
                       np.abs(rel).astype(np.float32)).astype(np.float32)
    log_pos = np.ceil(np.log(abs_pos / np.float32(mid)) / np.float32(np.log((max_position - 1) / mid))
                      * np.float32(mid - 1)) + np.float32(mid)
    bucket = np.where(abs_pos <= mid, rel.astype(np.float32), (log_pos * sign))
    return bucket.astype(np.int32)


def _msel_matrices():
    """Host-precomputed one-hot expansion matrices (bf16); numpy math verified
    bit-identical to the reference's jax _log_bucket."""
    e = np.arange(EW)
    d = 1023 - e                      # reversed distance axis
    bkt = _np_log_bucket(d)
    mc = np.clip(bkt + SPAN, 0, 2 * SPAN - 1)
    mp = np.clip(-bkt + SPAN, 0, 2 * SPAN - 1)
    mselk = np.zeros((K2, EW), BF16NP)
    mselk[mc, np.arange(EW)] = 1.0
    mselq = np.zeros((K2, EW), BF16NP)
    mselq[mp, np.arange(EW)] = 1.0
    return mselk, mselq


def _build():
    nc = bacc.Bacc(trn_type="TRN2", num_devices=NCORES, debug=False)

    def din(name, shape, dt=F32):
        return nc.dram_tensor(name, shape, dt, kind="ExternalInput")

    x_d = din("x", [S, D], F32R)
    wq_d, wk_d, wv_d = din("wq", [D, CH], F32R), din("wk", [D, CH], F32R), din("wv", [D, CH], F32R)
    bqs_d, bk_d, bv_d = din("bqs", [CH]), din("bk", [CH]), din("bv", [CH])
    wo_d, boh_d = din("wo", [CH, D], F32R), din("boh", [D])
    rel_d = din("rel", [K2, D])
    lng_d, lnb_d = din("lng", [D]), din("lnb", [D])
    wpk_d, wpq_d = din("wpk", [D, CH], F32R), din("wpq", [D, CH], F32R)
    bpk_d, bpq_d = din("bpk", [CH]), din("bpq", [CH])
    mselk_d, mselq_d = din("mselk", [K2, EW], BF16), din("mselq", [K2, EW], BF16)
    out_d = nc.dram_tensor("out", [S, D], F32, kind="ExternalOutput")

    # internal DRAM band tables (per head)
    c2pt_d = [nc.dram_tensor(f"c2pt{h}", [S, TW], BF16) for h in range(NH)]
    p2ct_d = [nc.dram_tensor(f"p2ct{h}", [S, TW], BF16) for h in range(NH)]

    def bcast_ap(t, n):
        return bass.AP(tensor=t, offset=0, ap=[[0, P], [1, n]])

    def band_ap(t, blk):
        # band[p, jj] = tbl[128*blk + p, 127 - p + jj]
        return bass.AP(tensor=t, offset=128 * blk * TW + 127, ap=[[TW - 1, P], [1, S]])

    r32 = lambda ap: ap.bitcast(F32R)

    with tile.TileContext(nc) as tc:
        with (
            tc.tile_pool(name="pers", bufs=1) as pers,
            tc.tile_pool(name="ppA", bufs=2, space="PSUM") as ppA,      # tag tbl: [128,512] f32
            tc.tile_pool(name="ppQK", bufs=2, space="PSUM") as ppQK,    # tag qk: [128,1024] f32
        ):
            id_f = pers.tile([P, P], F32, tag="idf", name="idf")
            masks.make_identity(nc, id_f[:])
            id_r = pers.tile([P, P], F32R, tag="idr", name="idr")
            nc.vector.tensor_copy(id_r[:], id_f[:])
            id_b = pers.tile([P, P], BF16, tag="idb", name="idb")
            masks.make_identity(nc, id_b[:])

            pkeb = [pers.tile([P, EW], BF16, tag=f"pke{m}", name=f"pke{m}") for m in range(4)]
            pqeb = [pers.tile([P, EW], BF16, tag=f"pqe{m}", name=f"pqe{m}") for m in range(4)]
            qTb = [pers.tile([P, S], BF16, tag=f"qT{m}", name=f"qT{m}") for m in range(4)]
            kTb = [pers.tile([P, S], BF16, tag=f"kT{m}", name=f"kT{m}") for m in range(4)]
            v_aug = [pers.tile([P, 65 * NH], BF16, tag=f"vaug{m}", name=f"vaug{m}") for m in range(8)]
            ctxTn = [pers.tile([P, S], F32R, tag=f"ctxTn{m}", name=f"ctxTn{m}") for m in range(4)]

            # =========== phase 0a/0b: rel_emb LN, transpose, pos proj, expansion ===========
            with tc.tile_pool(name="p0b", bufs=1) as w0b, \
                 tc.tile_pool(name="p0bs", bufs=2) as s0b, \
                 tc.tile_pool(name="ppTP", bufs=2, space="PSUM") as ppTP:
                reT = [w0b.tile([P, K2], F32R, tag=f"reT{i}", name=f"reT{i}") for i in range(8)]
                with tc.tile_pool(name="p0a", bufs=1) as w0a:
                    re_t = [w0a.tile([P, D], F32, tag=f"re{i}", name=f"re{i}") for i in range(4)]
                    scr = w0a.tile([P, D], F32, tag="lnscr", name="lnscr")
                    st1 = w0a.tile([P, 1], F32, tag="lnst1", name="lnst1")
                    st2 = w0a.tile([P, 1], F32, tag="lnst2", name="lnst2")
                    lng_sb = w0a.tile([P, D], F32, tag="lng", name="lng")
                    lnb_sb = w0a.tile([P, D], F32, tag="lnb", name="lnb")
                    nc.sync.dma_start(lng_sb[:], bcast_ap(lng_d, D))
                    nc.sync.dma_start(lnb_sb[:], bcast_ap(lnb_d, D))
                    for i in range(4):
                        nc.sync.dma_start(re_t[i][:], rel_d.ap()[128 * i:128 * (i + 1), :])
                        nc.vector.tensor_reduce(st1[:], re_t[i][:], AX.X, ALU.add, negate=True)
                        nc.vector.tensor_scalar_mul(st1[:], st1[:], 1.0 / D)
                        nc.scalar.activation(re_t[i][:], re_t[i][:], AF.Identity, bias=st1[:])
                        nc.scalar.activation(scr[:], re_t[i][:], AF.Square, accum_out=st2[:])
                        nc.vector.tensor_scalar(st2[:], st2[:], 1.0 / D, 1e-5, ALU.mult, ALU.add)
                        nc.scalar.activation(st2[:], st2[:], AF.Sqrt)
                        nc.vector.reciprocal(st2[:], st2[:])
                        nc.scalar.activation(re_t[i][:], re_t[i][:], AF.Copy, scale=st2[:])
                        nc.vector.tensor_mul(re_t[i][:], re_t[i][:], lng_sb[:])
                        nc.vector.tensor_add(re_t[i][:], re_t[i][:], lnb_sb[:])
                    for dt_ in range(8):
                        for kt in range(4):
                            ps = ppTP.tile([P, P], F32, tag="tp", name="tp")
                            nc.tensor.matmul(ps[:], re_t[kt][:, 128 * dt_:128 * (dt_ + 1)],
                                             id_f[:], is_transpose=True)
                            if kt % 2 == 0:
                                nc.scalar.activation(reT[dt_][:, 128 * kt:128 * (kt + 1)], ps[:], AF.Copy)
                            else:
                                nc.vector.tensor_copy(reT[dt_][:, 128 * kt:128 * (kt + 1)], ps[:])

                posk = [w0b.tile([P, CH], BF16, tag=f"posk{m}", name=f"posk{m}") for m in range(4)]
                posq = [w0b.tile([P, CH], BF16, tag=f"posq{m}", name=f"posq{m}") for m in range(4)]
                bpk_sb = w0b.tile([P, CH], F32, tag="bpk", name="bpk")
                bpq_sb = w0b.tile([P, CH], F32, tag="bpq", name="bpq")
                nc.sync.dma_start(bpk_sb[:], bcast_ap(bpk_d, CH))
                nc.sync.dma_start(bpq_sb[:], bcast_ap(bpq_d, CH))
                for w_d, bias_sb, dst in ((wpk_d, bpk_sb, posk), (wpq_d, bpq_sb, posq)):
                    for m in range(4):
                        ps = ppA.tile([P, 512], F32, tag="tbl", name="tbl")
                        for k in range(8):
                            wt = s0b.tile([P, CH], F32R, tag="wstream", name="wstream")
                            nc.sync.dma_start(wt[:], w_d.ap()[128 * k:128 * (k + 1), :])
                            nc.tensor.matmul(ps[:], reT[k][:, 128 * m:128 * (m + 1)], wt[:],
                                             start=(k == 0), stop=(k == 7))
                        nc.vector.tensor_add(dst[m][:], ps[:], bias_sb[:])

                for msel_d, src, dst, sc in ((mselk_d, posk, pkeb, SC_PK), (mselq_d, posq, pqeb, SC_PQ)):
                    msel_sb = [w0b.tile([P, EW], BF16, tag=f"msel{k}", name=f"msel{k}") for k in range(4)]
                    for k in range(4):
                        nc.sync.dma_start(msel_sb[k][:], msel_d.ap()[128 * k:128 * (k + 1), :])
                    for m in range(4):
                        for nc0 in range(0, EW, 512):
                            ps = ppA.tile([P, 512], F32, tag="tbl", name="tbl")
                            for k in range(4):
                                nc.tensor.matmul(ps[:], src[k][:, 128 * m:128 * (m + 1)],
                                                 msel_sb[k][:, nc0:nc0 + 512],
                                                 start=(k == 0), stop=(k == 3))
                            if (nc0 // 512) % 2 == 0:
                                nc.scalar.activation(dst[m][:, nc0:nc0 + 512], ps[:], AF.Copy, scale=sc)
                            else:
                                nc.vector.tensor_scalar_mul(dst[m][:, nc0:nc0 + 512], ps[:], sc)

            # =========== phase 0c: x transpose + q/k/v projections ===========
            with tc.tile_pool(name="p0c", bufs=1) as w0c, \
                 tc.tile_pool(name="p0cs", bufs=2) as s0c, \
                 tc.tile_pool(name="ppTP2", bufs=2, space="PSUM") as ppTP2:
                xT = [w0c.tile([P, S], F32R, tag=f"xT{i}", name=f"xT{i}") for i in range(8)]
                for st in range(8):
                    xt_in = s0c.tile([P, D], F32R, tag="xin", name="xin")
                    nc.sync.dma_start(xt_in[:], x_d.ap()[128 * st:128 * (st + 1), :])
                    for dt_ in range(8):
                        ps = ppTP2.tile([P, P], F32R, tag="tp", name="tp")
                        nc.tensor.matmul(ps[:], xt_in[:, 128 * dt_:128 * (dt_ + 1)],
                                         id_r[:], is_transpose=True)
                        if dt_ % 2 == 0:
                            nc.scalar.activation(xT[dt_][:, 128 * st:128 * (st + 1)], ps[:], AF.Copy)
                        else:
                            nc.vector.tensor_copy(xT[dt_][:, 128 * st:128 * (st + 1)], ps[:])

                bias_col = w0c.tile([P, 8], F32, tag="biascol", name="biascol")
                for w_d, bias_d, dst, sc, bc0 in ((wq_d, bqs_d, qTb, SC_Q, 0), (wk_d, bk_d, kTb, 1.0, 4)):
                    w_sb = [w0c.tile([P, CH], F32R, tag=f"wqk{k}", name=f"wqk{k}") for k in range(8)]
                    for k in range(8):
                        nc.sync.dma_start(w_sb[k][:], w_d.ap()[128 * k:128 * (k + 1), :])
                    for m in range(4):
                        nc.sync.dma_start(bias_col[:, bc0 + m:bc0 + m + 1],
                                          bias_d.ap()[128 * m:128 * (m + 1)].unsqueeze(1))
                        for nc0 in (0, 512):
                            ps = ppA.tile([P, 512], F32, tag="tbl", name="tbl")
                            for k in range(8):
                                nc.tensor.matmul(ps[:], w_sb[k][:, 128 * m:128 * (m + 1)],
                                                 xT[k][:, nc0:nc0 + 512], start=(k == 0), stop=(k == 7))
                            nc.scalar.activation(dst[m][:, nc0:nc0 + 512], ps[:], AF.Identity,
                                                 scale=sc, bias=bias_col[:, bc0 + m:bc0 + m + 1])

                bv_sb = w0c.tile([P, CH], F32, tag="bv", name="bv")
                nc.sync.dma_start(bv_sb[:], bcast_ap(bv_d, CH))
                wv_sb = [w0c.tile([P, CH], F32R, tag=f"wqk{k}", name=f"wqk{k}") for k in range(8)]
                for k in range(8):
                    nc.sync.dma_start(wv_sb[k][:], wv_d.ap()[128 * k:128 * (k + 1), :])
                for m in range(8):
                    ps = ppA.tile([P, 512], F32, tag="tbl", name="tbl")
                    for k in range(8):
                        nc.tensor.matmul(ps[:], xT[k][:, 128 * m:128 * (m + 1)], wv_sb[k][:],
                                         start=(k == 0), stop=(k == 7))
                    for h in range(NH):
                        nc.vector.tensor_add(v_aug[m][:, 65 * h:65 * h + 64], ps[:, 64 * h:64 * h + 64],
                                             bv_sb[:, 64 * h:64 * h + 64])
                        nc.gpsimd.memset(v_aug[m][:, 65 * h + 64:65 * h + 65], 1.0)

            # =========== phase 1: per-head attention ===========
            with (
                tc.tile_pool(name="bands", bufs=9) as bandp,
                tc.tile_pool(name="pbandp", bufs=2) as pbandp,
                tc.tile_pool(name="tblsb", bufs=4) as tblsbp,
                tc.tile_pool(name="probs", bufs=10) as probsp,
                tc.tile_pool(name="small", bufs=2) as smallp,
                tc.tile_pool(name="ppCTX", bufs=1, space="PSUM") as ppCTX,
            ):
                for h in range(NH):
                    hi, ho = h // 2, 64 * (h % 2)

                    # c2p tables (per s-block) -> DRAM -> diagonal band reads
                    for i in range(NB):
                        e0 = 896 - 128 * i
                        for ci, (coff, cw) in enumerate(CHUNKS):
                            ps = ppA.tile([P, 512], F32, tag="tbl", name="tbl")
                            nc.tensor.matmul(ps[:, :cw], qTb[hi][ho:ho + 64, 128 * i:128 * (i + 1)],
                                             pkeb[hi][ho:ho + 64, e0 + coff:e0 + coff + cw])
                            tsb = tblsbp.tile([P, 512], BF16, tag="tsb", name="tsb")
                            if ci % 2 == 0:
                                nc.scalar.activation(tsb[:, :cw], ps[:, :cw], AF.Copy)
                            else:
                                nc.vector.tensor_copy(tsb[:, :cw], ps[:, :cw])
                            nc.sync.dma_start(c2pt_d[h].ap()[128 * i:128 * (i + 1), coff:coff + cw],
                                              tsb[:, :cw])
                    cband = []
                    for i in range(NB):
                        bt = bandp.tile([P, S], F32, tag="cband", name="cband")
                        nc.gpsimd.dma_start(bt[:], band_ap(c2pt_d[h], i))
                        cband.append(bt)

                    probs = []
                    for jb in range(NB):
                        e0 = 896 - 128 * jb
                        for ci, (coff, cw) in enumerate(CHUNKS):
                            ps = ppA.tile([P, 512], F32, tag="tbl", name="tbl")
                            nc.tensor.matmul(ps[:, :cw], kTb[hi][ho:ho + 64, 128 * jb:128 * (jb + 1)],
                                             pqeb[hi][ho:ho + 64, e0 + coff:e0 + coff + cw])
                            tsb = tblsbp.tile([P, 512], BF16, tag="tsb", name="tsb")
                            if ci % 2 == 0:
                                nc.vector.tensor_copy(tsb[:, :cw], ps[:, :cw])
                            else:
                                nc.scalar.activation(tsb[:, :cw], ps[:, :cw], AF.Copy)
                            nc.sync.dma_start(p2ct_d[h].ap()[128 * jb:128 * (jb + 1), coff:coff + cw],
                                              tsb[:, :cw])
                        pband = pbandp.tile([P, S], BF16, tag="pband", name="pband")
                        nc.sync.dma_start(pband[:], band_ap(p2ct_d[h], jb))

                        sc_ps = ppQK.tile([P, S], F32, tag="qk", name="qk")
                        for nc0 in (0, 512):
                            nc.tensor.matmul(sc_ps[:, nc0:nc0 + 512],
                                             kTb[hi][ho:ho + 64, 128 * jb:128 * (jb + 1)],
                                             qTb[hi][ho:ho + 64, nc0:nc0 + 512], start=True, stop=False)
                        for i in range(NB):
                            nc.tensor.matmul(sc_ps[:, 128 * i:128 * (i + 1)],
                                             cband[i][:, 128 * jb:128 * (jb + 1)], id_f[:],
                                             is_transpose=True, start=False, stop=False)
                        for nc0 in (0, 512):
                            nc.tensor.matmul(sc_ps[:, nc0:nc0 + 512], id_b[:], pband[:, nc0:nc0 + 512],
                                             start=False, stop=True)
                        pt = probsp.tile([P, S], BF16, tag="probsT", name="probsT")
                        nc.scalar.activation(pt[:], sc_ps[:], AF.Exp)
                        probs.append(pt)

                    cps = ppCTX.tile([P, S], F32, tag="ctx", name="ctx")
                    for jb in range(NB):
                        for nc0 in (0, 512):
                            nc.tensor.matmul(cps[0:65, nc0:nc0 + 512], v_aug[jb][:, 65 * h:65 * h + 65],
                                             probs[jb][:, nc0:nc0 + 512], start=(jb == 0), stop=(jb == 7))
                    rd = smallp.tile([1, S], F32, tag="recip", name="recip")
                    rdb = smallp.tile([64, S], F32, tag="recipb", name="recipb")
                    nc.vector.reciprocal(rd[:], cps[64:65, :])
                    nc.gpsimd.partition_broadcast(rdb[:], rd[:])
                    nc.vector.tensor_mul(ctxTn[hi][ho:ho + 64, :], cps[0:64, :], rdb[:])

            # =========== phase 2: output projection ===========
            with tc.tile_pool(name="p2", bufs=1) as w2, tc.tile_pool(name="p2s", bufs=2) as s2:
                wo_sb = [w2.tile([P, D], F32R, tag=f"wo{k}", name=f"wo{k}") for k in range(4)]
                for k in range(4):
                    nc.sync.dma_start(wo_sb[k][:], wo_d.ap()[128 * k:128 * (k + 1), :])
                bo_sb = w2.tile([P, D], F32, tag="bo", name="bo")
                nc.sync.dma_start(bo_sb[:], bcast_ap(boh_d, D))
                for m in range(8):
                    ps = ppQK.tile([P, S], F32, tag="qk", name="qk")
                    for nc0 in (0, 512):
                        for k in range(4):
                            nc.tensor.matmul(ps[:, nc0:nc0 + 512], ctxTn[k][:, 128 * m:128 * (m + 1)],
                                             wo_sb[k][:, nc0:nc0 + 512], start=(k == 0), stop=(k == 3))
                    osb = s2.tile([P, D], F32, tag="osb", name="osb")
                    nc.vector.tensor_add(osb[:], ps[:], bo_sb[:])
                    nc.sync.dma_start(out_d.ap()[128 * m:128 * (m + 1), :], osb[:])

    nc.compile()
    return nc


_NC = None


def _prep_in_maps(hidden_states, Wq, bq, Wk, bk, Wv, bv, Wo, bo,
                  rel_emb, ln_g, ln_b, Wpk, bpk, Wpq, bpq):
    mselk, mselq = _msel_matrices()
    f32 = lambda a: np.ascontiguousarray(np.asarray(a, dtype=np.float32))

    in_maps = []
    for core in range(NCORES):
        b, g = core // 2, core % 2
        hs = slice(CH * g, CH * (g + 1))
        in_maps.append({
            "x": f32(hidden_states[b]),
            "wq": f32(np.asarray(Wq)[:, hs]), "wk": f32(np.asarray(Wk)[:, hs]),
            "wv": f32(np.asarray(Wv)[:, hs]),
            "bqs": f32(np.asarray(bq)[hs] * SC_Q), "bk": f32(np.asarray(bk)[hs]),
            "bv": f32(np.asarray(bv)[hs]),
            "wo": f32(np.asarray(Wo)[hs, :]),
            "boh": f32(bo) if g == 0 else np.zeros(D, np.float32),
            "rel": f32(rel_emb), "lng": f32(ln_g), "lnb": f32(ln_b),
            "wpk": f32(np.asarray(Wpk)[:, hs]), "wpq": f32(np.asarray(Wpq)[:, hs]),
            "bpk": f32(np.asarray(bpk)[hs]), "bpq": f32(np.asarray(bpq)[hs]),
            "mselk": mselk, "mselq": mselq,
        })

    return in_maps


def _run(in_maps, trace=False):
    global _NC
    if _NC is None:
        _NC = _build()
    res = run_bass_kernel_spmd(_NC, in_maps, core_ids=list(range(NCORES)), trace=trace)
    out = np.stack([
        res.results[2 * b]["out"].astype(np.float32) + res.results[2 * b + 1]["out"].astype(np.float32)
        for b in range(B)
    ])
    return out, res


def kernel(**inputs):
    out, _ = _run(_prep_in_maps(**inputs))
    return out


def kernel_traced(**inputs):
    in_maps = _prep_in_maps(**inputs)
    try:
        out, res = _run(in_maps, trace=True)
    except ModuleNotFoundError:
        out, res = _run(in_maps, trace=False)
    return out, res



# revision 7
# speedup vs baseline: 1.2608x; 1.2608x over previous
"""DisentangledSelfAttention (DeBERTa-style) Trainium2 Bass kernel.

Sharding: 8 cores = 4 batches x 2 head-halves. Each core computes one batch
with 8 heads; the output projection is row-parallel, partial outputs are
summed pairwise on the host (unshard step).

Core algorithm notes:
 - scores are assembled TRANSPOSED (scoresT[j, s]) so softmax normalization
   folds into the ctx matmul via an appended ones-column (row 64 of ctxT
   psum = denominator) and probsT feeds the ctx matmul with no transpose.
 - the rel-position gathers (c2p/p2c) become diagonal "band" reads of
   expanded tables. Expansion happens on the PE via one-hot matmuls with
   host-precomputed Msel matrices; tables round-trip through DRAM so the
   diagonal read is a flat (affine) DRAM access pattern. The expansion axis
   is reversed (d = 1023 - E) so band reads are contiguous.
 - v2: bf16 weights/activations throughout (projection weights cast on
   host), software-pipelined head loop (tables of head h+1 are emitted
   ahead of attention of head h so the PE never drains), consolidated DMA
   instructions, and psum drains spread across scalar/vector/gpsimd.
"""

import os
import sys

import numpy as np

for _p in ("/opt/trn_rl_repo", "/root/.axon_site/_ro/trn_rl_repo"):
    if os.path.isdir(_p) and _p not in sys.path:
        sys.path.insert(0, _p)

import ml_dtypes  # noqa: E402
import concourse.bass as bass  # noqa: E402
import concourse.bacc as bacc  # noqa: E402
import concourse.mybir as mybir  # noqa: E402
import concourse.tile as tile  # noqa: E402
import concourse.masks as masks  # noqa: E402
from concourse.bass_utils import run_bass_kernel_spmd  # noqa: E402

F32 = mybir.dt.float32
F32R = mybir.dt.float32r
BF16 = mybir.dt.bfloat16
AF = mybir.ActivationFunctionType
ALU = mybir.AluOpType
AX = mybir.AxisListType
BF16NP = ml_dtypes.bfloat16

B, S, D, H, HD = 4, 1024, 1024, 16, 64
NCORES = 8
NH = 8          # heads per core
CH = 512        # feature columns per core
SPAN, K2 = 256, 512
EW = 2048       # expanded table width (global E axis; d = 1023 - E)
TW = 1152       # per-block table window width
NB = 8          # 128-row blocks in S
P = 128

SC_Q = float(1.0 / np.sqrt(HD * 3))     # folded into qT
SC_PK = float(np.sqrt(1.5))             # c2p table extra scale (qT carries 1/sqrt(192))
SC_PQ = float(1.0 / np.sqrt(HD * 2))    # p2c table scale
CHUNKS = ((0, 512), (512, 512), (1024, 128))  # TW split into psum-bank matmuls


def _np_log_bucket(rel, bucket_size=SPAN, max_position=512):
    rel = np.asarray(rel)
    mid = bucket_size // 2
    sign = np.sign(rel).astype(np.float32)
    abs_pos = np.where((rel < mid) & (rel > -mid), np.float32(mid - 1),
                       np.abs(rel).astype(np.float32)).astype(np.float32)
    log_pos = np.ceil(np.log(abs_pos / np.float32(mid)) / np.float32(np.log((max_position - 1) / mid))
                      * np.float32(mid - 1)) + np.float32(mid)
    bucket = np.where(abs_pos <= mid, rel.astype(np.float32), (log_pos * sign))
    return bucket.astype(np.int32)


def _msel_matrices():
    """Host-precomputed one-hot expansion matrices (bf16); numpy math verified
    bit-identical to the reference's jax _log_bucket."""
    e = np.arange(EW)
    d = 1023 - e                      # reversed distance axis
    bkt = _np_log_bucket(d)
    mc = np.clip(bkt + SPAN, 0, 2 * SPAN - 1)
    mp = np.clip(-bkt + SPAN, 0, 2 * SPAN - 1)
    mselk = np.zeros((K2, EW), BF16NP)
    mselk[mc, np.arange(EW)] = 1.0
    mselq = np.zeros((K2, EW), BF16NP)
    mselq[mp, np.arange(EW)] = 1.0
    return mselk, mselq


def _build():
    nc = bacc.Bacc(trn_type="TRN2", num_devices=NCORES, debug=False)

    def din(name, shape, dt=F32):
        return nc.dram_tensor(name, shape, dt, kind="ExternalInput")

    x_d = din("x", [S, D], F32R)
    wq_d, wk_d, wv_d = din("wq", [D, CH], BF16), din("wk", [D, CH], BF16), din("wv", [D, CH], BF16)
    bqs_d, bk_d, bv_d = din("bqs", [CH]), din("bk", [CH]), din("bv", [CH])
    wo_d, boh_d = din("wo", [CH, D], BF16), din("boh", [D])
    rel_d = din("rel", [K2, D])
    lng_d, lnb_d = din("lng", [D]), din("lnb", [D])
    wpk_d, wpq_d = din("wpk", [D, CH], BF16), din("wpq", [D, CH], BF16)
    bpk_d, bpq_d = din("bpk", [CH]), din("bpq", [CH])
    mselk_d, mselq_d = din("mselk", [K2, EW], BF16), din("mselq", [K2, EW], BF16)
    out_d = nc.dram_tensor("out", [S, D], F32, kind="ExternalOutput")

    # internal DRAM band tables (per head)
    c2pt_d = [nc.dram_tensor(f"c2pt{h}", [S, TW], BF16) for h in range(NH)]
    p2ct_d = [nc.dram_tensor(f"p2ct{h}", [S, TW], BF16) for h in range(NH)]

    def bcast_ap(t, n):
        return bass.AP(tensor=t, offset=0, ap=[[0, P], [1, n]])

    def band_ap(t, blk):
        # band[p, jj] = tbl[128*blk + p, 127 - p + jj]
        return bass.AP(tensor=t, offset=128 * blk * TW + 127, ap=[[TW - 1, P], [1, S]])

    def band_ap4(t, g):
        # 4-block grouped band read: out[p, 1024*b + jj] = tbl[128*(4g+b) + p, 127 - p + jj]
        return bass.AP(tensor=t, offset=128 * (4 * g) * TW + 127,
                       ap=[[TW - 1, P], [128 * TW, 4], [1, S]])

    with tile.TileContext(nc) as tc:
        with (
            tc.tile_pool(name="pers", bufs=1) as pers,
            tc.tile_pool(name="ppT", bufs=2, space="PSUM") as ppT,      # tag tbl: [128,512] f32
            tc.tile_pool(name="ppS", bufs=2, space="PSUM") as ppS,      # tag qk: [128,1024] f32
        ):
            id_f = pers.tile([P, P], F32, tag="idf", name="idf")
            masks.make_identity(nc, id_f[:])
            id_r = pers.tile([P, P], F32R, tag="idr", name="idr")
            nc.vector.tensor_copy(id_r[:], id_f[:])
            id_b = pers.tile([P, P], BF16, tag="idb", name="idb")
            masks.make_identity(nc, id_b[:])

            pkeb = [pers.tile([P, EW], BF16, tag=f"pke{m}", name=f"pke{m}") for m in range(4)]
            pqeb = [pers.tile([P, EW], BF16, tag=f"pqe{m}", name=f"pqe{m}") for m in range(4)]
            qTb = [pers.tile([P, S], BF16, tag=f"qT{m}", name=f"qT{m}") for m in range(4)]
            kTb = [pers.tile([P, S], BF16, tag=f"kT{m}", name=f"kT{m}") for m in range(4)]
            v_aug = [pers.tile([P, 65 * NH], BF16, tag=f"vaug{m}", name=f"vaug{m}") for m in range(8)]
            ctxTn = [pers.tile([P, S], BF16, tag=f"ctxTn{m}", name=f"ctxTn{m}") for m in range(4)]

            # =========== phase 0a/0b: rel_emb LN, transpose, pos proj, expansion ===========
            with tc.tile_pool(name="p0b", bufs=1) as w0b, \
                 tc.tile_pool(name="p0bs", bufs=2) as s0b, \
                 tc.tile_pool(name="ppTP", bufs=2, space="PSUM") as ppTP:
                reT = [w0b.tile([P, K2], BF16, tag=f"reT{i}", name=f"reT{i}") for i in range(8)]
                with tc.tile_pool(name="p0a", bufs=1) as w0a:
                    re_t = [w0a.tile([P, D], F32, tag=f"re{i}", name=f"re{i}") for i in range(4)]
                    scr = w0a.tile([P, D], F32, tag="lnscr", name="lnscr")
                    st1 = w0a.tile([P, 1], F32, tag="lnst1", name="lnst1")
                    st2 = w0a.tile([P, 1], F32, tag="lnst2", name="lnst2")
                    lng_sb = w0a.tile([P, D], F32, tag="lng", name="lng")
                    lnb_sb = w0a.tile([P, D], F32, tag="lnb", name="lnb")
                    nc.sync.dma_start(lng_sb[:], bcast_ap(lng_d, D))
                    nc.sync.dma_start(lnb_sb[:], bcast_ap(lnb_d, D))
                    for i in range(4):
                        nc.sync.dma_start(re_t[i][:], rel_d.ap()[128 * i:128 * (i + 1), :])
                        nc.vector.tensor_reduce(st1[:], re_t[i][:], AX.X, ALU.add, negate=True)
                        nc.vector.tensor_scalar_mul(st1[:], st1[:], 1.0 / D)
                        nc.scalar.activation(re_t[i][:], re_t[i][:], AF.Identity, bias=st1[:])
                        nc.scalar.activation(scr[:], re_t[i][:], AF.Square, accum_out=st2[:])
                        nc.vector.tensor_scalar(st2[:], st2[:], 1.0 / D, 1e-5, ALU.mult, ALU.add)
                        nc.scalar.activation(st2[:], st2[:], AF.Sqrt)
                        nc.vector.reciprocal(st2[:], st2[:])
                        nc.scalar.activation(re_t[i][:], re_t[i][:], AF.Copy, scale=st2[:])
                        nc.vector.tensor_mul(re_t[i][:], re_t[i][:], lng_sb[:])
                        nc.vector.tensor_add(re_t[i][:], re_t[i][:], lnb_sb[:])
                    for dt_ in range(8):
                        for kt in range(4):
                            ps = ppTP.tile([P, P], F32, tag="tp", name="tp")
                            nc.tensor.matmul(ps[:], re_t[kt][:, 128 * dt_:128 * (dt_ + 1)],
                                             id_f[:], is_transpose=True)
                            if kt % 2 == 0:
                                nc.scalar.activation(reT[dt_][:, 128 * kt:128 * (kt + 1)], ps[:], AF.Copy)
                            else:
                                nc.vector.tensor_copy(reT[dt_][:, 128 * kt:128 * (kt + 1)], ps[:])

                posk = [w0b.tile([P, CH], BF16, tag=f"posk{m}", name=f"posk{m}") for m in range(4)]
                posq = [w0b.tile([P, CH], BF16, tag=f"posq{m}", name=f"posq{m}") for m in range(4)]
                bpk_sb = w0b.tile([P, CH], F32, tag="bpk", name="bpk")
                bpq_sb = w0b.tile([P, CH], F32, tag="bpq", name="bpq")
                nc.sync.dma_start(bpk_sb[:], bcast_ap(bpk_d, CH))
                nc.sync.dma_start(bpq_sb[:], bcast_ap(bpq_d, CH))
                for w_d, bias_sb, dst in ((wpk_d, bpk_sb, posk), (wpq_d, bpq_sb, posq)):
                    for m in range(4):
                        ps = ppT.tile([P, 512], F32, tag="tbl", name="tbl")
                        for k in range(8):
                            wt = s0b.tile([P, CH], BF16, tag="wstream", name="wstream")
                            nc.sync.dma_start(wt[:], w_d.ap()[128 * k:128 * (k + 1), :])
                            nc.tensor.matmul(ps[:], reT[k][:, 128 * m:128 * (m + 1)], wt[:],
                                             start=(k == 0), stop=(k == 7))
                        nc.vector.tensor_add(dst[m][:], ps[:], bias_sb[:])

                for msel_d, src, dst, sc in ((mselk_d, posk, pkeb, SC_PK), (mselq_d, posq, pqeb, SC_PQ)):
                    msel_sb = [w0b.tile([P, EW], BF16, tag=f"msel{k}", name=f"msel{k}") for k in range(4)]
                    for k in range(4):
                        nc.sync.dma_start(msel_sb[k][:], msel_d.ap()[128 * k:128 * (k + 1), :])
                    for m in range(4):
                        for nc0 in range(0, EW, 512):
                            ps = ppT.tile([P, 512], F32, tag="tbl", name="tbl")
                            for k in range(4):
                                nc.tensor.matmul(ps[:], src[k][:, 128 * m:128 * (m + 1)],
                                                 msel_sb[k][:, nc0:nc0 + 512],
                                                 start=(k == 0), stop=(k == 3))
                            if (nc0 // 512) % 2 == 0:
                                nc.scalar.activation(dst[m][:, nc0:nc0 + 512], ps[:], AF.Copy, scale=sc)
                            else:
                                nc.vector.tensor_scalar_mul(dst[m][:, nc0:nc0 + 512], ps[:], sc)

            # =========== phase 0c: x transpose + q/k/v projections ===========
            with tc.tile_pool(name="p0c", bufs=1) as w0c, \
                 tc.tile_pool(name="p0cs", bufs=2) as s0c, \
                 tc.tile_pool(name="ppTP2", bufs=2, space="PSUM") as ppTP2:
                xT = [w0c.tile([P, S], BF16, tag=f"xT{i}", name=f"xT{i}") for i in range(8)]
                for st in range(8):
                    xt_in = s0c.tile([P, D], F32R, tag="xin", name="xin")
                    nc.sync.dma_start(xt_in[:], x_d.ap()[128 * st:128 * (st + 1), :])
                    for dt_ in range(8):
                        ps = ppTP2.tile([P, P], F32R, tag="tp", name="tp")
                        nc.tensor.matmul(ps[:], xt_in[:, 128 * dt_:128 * (dt_ + 1)],
                                         id_r[:], is_transpose=True)
                        if dt_ % 2 == 0:
                            nc.scalar.activation(xT[dt_][:, 128 * st:128 * (st + 1)],
                                                 ps[:].bitcast(F32), AF.Copy)
                        else:
                            nc.vector.tensor_copy(xT[dt_][:, 128 * st:128 * (st + 1)],
                                                  ps[:].bitcast(F32))

                bias_col = w0c.tile([P, 8], F32, tag="biascol", name="biascol")
                for w_d, bias_d, dst, sc, bc0 in ((wq_d, bqs_d, qTb, SC_Q, 0), (wk_d, bk_d, kTb, 1.0, 4)):
                    w_sb = [w0c.tile([P, CH], BF16, tag=f"wqk{k}", name=f"wqk{k}") for k in range(8)]
                    for k in range(8):
                        nc.sync.dma_start(w_sb[k][:], w_d.ap()[128 * k:128 * (k + 1), :])
                    for m in range(4):
                        nc.sync.dma_start(bias_col[:, bc0 + m:bc0 + m + 1],
                                          bias_d.ap()[128 * m:128 * (m + 1)].unsqueeze(1))
                        for nc0 in (0, 512):
                            ps = ppT.tile([P, 512], F32, tag="tbl", name="tbl")
                            for k in range(8):
                                nc.tensor.matmul(ps[:], w_sb[k][:, 128 * m:128 * (m + 1)],
                                                 xT[k][:, nc0:nc0 + 512], start=(k == 0), stop=(k == 7))
                            nc.scalar.activation(dst[m][:, nc0:nc0 + 512], ps[:], AF.Identity,
                                                 scale=sc, bias=bias_col[:, bc0 + m:bc0 + m + 1])

                bv_sb = w0c.tile([P, CH], F32, tag="bv", name="bv")
                nc.sync.dma_start(bv_sb[:], bcast_ap(bv_d, CH))
                wv_sb = [w0c.tile([P, CH], BF16, tag=f"wqk{k}", name=f"wqk{k}") for k in range(8)]
                for k in range(8):
                    nc.sync.dma_start(wv_sb[k][:], wv_d.ap()[128 * k:128 * (k + 1), :])
                for m in range(8):
                    ps = ppT.tile([P, 512], F32, tag="tbl", name="tbl")
                    for k in range(8):
                        nc.tensor.matmul(ps[:], xT[k][:, 128 * m:128 * (m + 1)], wv_sb[k][:],
                                         start=(k == 0), stop=(k == 7))
                    for h in range(NH):
                        nc.vector.tensor_add(v_aug[m][:, 65 * h:65 * h + 64], ps[:, 64 * h:64 * h + 64],
                                             bv_sb[:, 64 * h:64 * h + 64])
                        nc.gpsimd.memset(v_aug[m][:, 65 * h + 64:65 * h + 65], 1.0)

            # =========== phase 1: per-head attention (software-pipelined) ===========
            with (
                tc.tile_pool(name="bands", bufs=4) as bandp,
                tc.tile_pool(name="pbandp", bufs=10) as pbandp,
                tc.tile_pool(name="tblsb", bufs=6) as tblsbp,
                tc.tile_pool(name="probs", bufs=10) as probsp,
                tc.tile_pool(name="small", bufs=2) as smallp,
                tc.tile_pool(name="ppCTX", bufs=1, space="PSUM") as ppCTX,
            ):
                drain_rr = [nc.vector, nc.scalar]

                def emit_tables(h):
                    """c2p + p2c block tables -> DRAM; grouped cband reads.
                    Returns (cband_groups, None); pband reads are issued in emit_attn."""
                    hi, ho = h // 2, 64 * (h % 2)
                    cband_g = []
                    for tbl_i, (src, dst_d, sc_done) in enumerate(
                            ((pkeb, c2pt_d[h], None), (pqeb, p2ct_d[h], None))):
                        for i in range(NB):
                            e0 = 896 - 128 * i
                            tsb = tblsbp.tile([P, TW], BF16, tag="tsb", name="tsb")
                            for ci, (coff, cw) in enumerate(CHUNKS):
                                ps = ppT.tile([P, 512], F32, tag="tbl", name="tbl")
                                qk_src = qTb if tbl_i == 0 else kTb
                                nc.tensor.matmul(ps[:, :cw], qk_src[hi][ho:ho + 64, 128 * i:128 * (i + 1)],
                                                 src[hi][ho:ho + 64, e0 + coff:e0 + coff + cw])
                                eng = drain_rr[(i * 3 + ci) % 2]
                                if eng is nc.scalar:
                                    nc.scalar.activation(tsb[:, coff:coff + cw], ps[:, :cw], AF.Copy)
                                else:
                                    eng.tensor_copy(tsb[:, coff:coff + cw], ps[:, :cw])
                            nc.sync.dma_start(dst_d.ap()[128 * i:128 * (i + 1), :], tsb[:])
                            if tbl_i == 0 and i % 4 == 3:
                                bt = bandp.tile([P, 4 * S], F32, tag="cband", name="cband")
                                nc.gpsimd.dma_start(bt[:], band_ap4(c2pt_d[h], i // 4))
                                cband_g.append(bt)
                    return cband_g

                def emit_attn(h, cband_g):
                    hi, ho = h // 2, 64 * (h % 2)
                    pbands = []
                    for jb in range(NB):
                        pband = pbandp.tile([P, S], BF16, tag="pband", name="pband")
                        nc.gpsimd.dma_start(pband[:], band_ap(p2ct_d[h], jb))
                        pbands.append(pband)

                    probs = []
                    for jb in range(NB):
                        sc_ps = ppS.tile([P, S], F32, tag="qk", name="qk")
                        for nc0 in (0, 512):
                            nc.tensor.matmul(sc_ps[:, nc0:nc0 + 512],
                                             kTb[hi][ho:ho + 64, 128 * jb:128 * (jb + 1)],
                                             qTb[hi][ho:ho + 64, nc0:nc0 + 512], start=True, stop=False)
                        for i in range(NB):
                            nc.tensor.matmul(sc_ps[:, 128 * i:128 * (i + 1)],
                                             cband_g[i // 4][:, S * (i % 4) + 128 * jb:S * (i % 4) + 128 * (jb + 1)],
                                             id_f[:], is_transpose=True, start=False, stop=False)
                        for nc0 in (0, 512):
                            nc.tensor.matmul(sc_ps[:, nc0:nc0 + 512], id_b[:],
                                             pbands[jb][:, nc0:nc0 + 512],
                                             start=False, stop=True)
                        pt = probsp.tile([P, S], BF16, tag="probsT", name="probsT")
                        nc.scalar.activation(pt[:], sc_ps[:], AF.Exp)
                        probs.append(pt)

                    cps = ppCTX.tile([P, S], F32, tag="ctx", name="ctx")
                    for jb in range(NB):
                        for nc0 in (0, 512):
                            nc.tensor.matmul(cps[0:65, nc0:nc0 + 512], v_aug[jb][:, 65 * h:65 * h + 65],
                                             probs[jb][:, nc0:nc0 + 512], start=(jb == 0), stop=(jb == 7))
                    rd = smallp.tile([1, S], F32, tag="recip", name="recip")
                    rdb = smallp.tile([64, S], F32, tag="recipb", name="recipb")
                    nc.vector.reciprocal(rd[:], cps[64:65, :])
                    nc.gpsimd.partition_broadcast(rdb[:], rd[:])
                    nc.vector.tensor_mul(ctxTn[hi][ho:ho + 64, :], cps[0:64, :], rdb[:])

                prev = emit_tables(0)
                for h in range(1, NH):
                    cur = emit_tables(h)
                    emit_attn(h - 1, prev)
                    prev = cur
                emit_attn(NH - 1, prev)

            # =========== phase 2: output projection ===========
            with tc.tile_pool(name="p2", bufs=1) as w2, tc.tile_pool(name="p2s", bufs=2) as s2:
                wo_sb = [w2.tile([P, D], BF16, tag=f"wo{k}", name=f"wo{k}") for k in range(4)]
                for k in range(4):
                    nc.sync.dma_start(wo_sb[k][:], wo_d.ap()[128 * k:128 * (k + 1), :])
                bo_sb = w2.tile([P, D], F32, tag="bo", name="bo")
                nc.sync.dma_start(bo_sb[:], bcast_ap(boh_d, D))
                for m in range(8):
                    ps = ppS.tile([P, S], F32, tag="qk", name="qk")
                    for nc0 in (0, 512):
                        for k in range(4):
                            nc.tensor.matmul(ps[:, nc0:nc0 + 512], ctxTn[k][:, 128 * m:128 * (m + 1)],
                                             wo_sb[k][:, nc0:nc0 + 512], start=(k == 0), stop=(k == 3))
                    osb = s2.tile([P, D], F32, tag="osb", name="osb")
                    nc.vector.tensor_add(osb[:], ps[:], bo_sb[:])
                    nc.sync.dma_start(out_d.ap()[128 * m:128 * (m + 1), :], osb[:])

    nc.compile()
    return nc


_NC = None


def _prep_in_maps(hidden_states, Wq, bq, Wk, bk, Wv, bv, Wo, bo,
                  rel_emb, ln_g, ln_b, Wpk, bpk, Wpq, bpq):
    mselk, mselq = _msel_matrices()
    f32 = lambda a: np.ascontiguousarray(np.asarray(a, dtype=np.float32))
    b16 = lambda a: np.ascontiguousarray(np.asarray(a, dtype=np.float32).astype(BF16NP))

    in_maps = []
    for core in range(NCORES):
        b, g = core // 2, core % 2
        hs = slice(CH * g, CH * (g + 1))
        in_maps.append({
            "x": f32(hidden_states[b]),
            "wq": b16(np.asarray(Wq)[:, hs]), "wk": b16(np.asarray(Wk)[:, hs]),
            "wv": b16(np.asarray(Wv)[:, hs]),
            "bqs": f32(np.asarray(bq)[hs] * SC_Q), "bk": f32(np.asarray(bk)[hs]),
            "bv": f32(np.asarray(bv)[hs]),
            "wo": b16(np.asarray(Wo)[hs, :]),
            "boh": f32(bo) if g == 0 else np.zeros(D, np.float32),
            "rel": f32(rel_emb), "lng": f32(ln_g), "lnb": f32(ln_b),
            "wpk": b16(np.asarray(Wpk)[:, hs]), "wpq": b16(np.asarray(Wpq)[:, hs]),
            "bpk": f32(np.asarray(bpk)[hs]), "bpq": f32(np.asarray(bpq)[hs]),
            "mselk": mselk, "mselq": mselq,
        })

    return in_maps


def _run(in_maps, trace=False):
    global _NC
    if _NC is None:
        _NC = _build()
    res = run_bass_kernel_spmd(_NC, in_maps, core_ids=list(range(NCORES)), trace=trace)
    out = np.stack([
        res.results[2 * b]["out"].astype(np.float32) + res.results[2 * b + 1]["out"].astype(np.float32)
        for b in range(B)
    ])
    return out, res


def kernel(**inputs):
    out, _ = _run(_prep_in_maps(**inputs))
    return out


def kernel_traced(**inputs):
    in_maps = _prep_in_maps(**inputs)
    try:
        out, res = _run(in_maps, trace=True)
    except ModuleNotFoundError:
        out, res = _run(in_maps, trace=False)
    return out, res


# revision 13
# speedup vs baseline: 1.4128x; 1.1206x over previous
"""DisentangledSelfAttention (DeBERTa-style) Trainium2 Bass kernel.

Sharding: 8 cores = 4 batches x 2 head-halves. Each core computes one batch
with 8 heads; the output projection is row-parallel, partial outputs are
summed pairwise on the host (unshard step).

Core algorithm notes:
 - scores are assembled TRANSPOSED (scoresT[j, s]) so softmax normalization
   folds into the ctx matmul via an appended ones-column (row 64 of ctxT
   psum = denominator) and probsT feeds the ctx matmul with no transpose.
 - the rel-position gathers (c2p/p2c) become diagonal "band" reads of
   expanded tables. Expansion happens on the PE via one-hot matmuls with
   host-precomputed Msel matrices; tables round-trip through DRAM so the
   diagonal read is a flat (affine) DRAM access pattern. The expansion axis
   is reversed (d = 1023 - E) so band reads are contiguous.
 - v3: bf16 weights/activations (cast on host), one consolidated DMA per
   input tensor, x-transpose + q/k/v projections emitted first so the PE
   fills while the rel-embedding LN chain resolves, software-pipelined head
   loop (tables of head h+1 ahead of attention of head h), psum drains
   balanced across scalar/vector, normalization via DVE divide.
"""

import os
import sys

import numpy as np

for _p in ("/opt/trn_rl_repo", "/root/.axon_site/_ro/trn_rl_repo"):
    if os.path.isdir(_p) and _p not in sys.path:
        sys.path.insert(0, _p)

import ml_dtypes  # noqa: E402
import concourse.bass as bass  # noqa: E402
import concourse.bacc as bacc  # noqa: E402
import concourse.mybir as mybir  # noqa: E402
import concourse.tile as tile  # noqa: E402
import concourse.masks as masks  # noqa: E402
from concourse.bass_utils import run_bass_kernel_spmd  # noqa: E402

F32 = mybir.dt.float32
F32R = mybir.dt.float32r
BF16 = mybir.dt.bfloat16
AF = mybir.ActivationFunctionType
ALU = mybir.AluOpType
AX = mybir.AxisListType
BF16NP = ml_dtypes.bfloat16

B, S, D, H, HD = 4, 1024, 1024, 16, 64
NCORES = 8
NH = 8          # heads per core
CH = 512        # feature columns per core
SPAN, K2 = 256, 512
EW = 2048       # expanded table width (global E axis; d = 1023 - E)
TW = 1152       # per-block table window width
NB = 8          # 128-row blocks in S
P = 128

SC_Q = float(1.0 / np.sqrt(HD * 3))     # folded into qT
SC_PK = float(np.sqrt(1.5))             # c2p table extra scale (qT carries 1/sqrt(192))
SC_PQ = float(1.0 / np.sqrt(HD * 2))    # p2c table scale
CHUNKS = ((0, 512), (512, 512), (1024, 128))  # TW split into psum-bank matmuls


def _np_log_bucket(rel, bucket_size=SPAN, max_position=512):
    rel = np.asarray(rel)
    mid = bucket_size // 2
    sign = np.sign(rel).astype(np.float32)
    abs_pos = np.where((rel < mid) & (rel > -mid), np.float32(mid - 1),
                       np.abs(rel).astype(np.float32)).astype(np.float32)
    log_pos = np.ceil(np.log(abs_pos / np.float32(mid)) / np.float32(np.log((max_position - 1) / mid))
                      * np.float32(mid - 1)) + np.float32(mid)
    bucket = np.where(abs_pos <= mid, rel.astype(np.float32), (log_pos * sign))
    return bucket.astype(np.int32)


def _msel_matrices():
    """Host-precomputed one-hot expansion matrices (bf16); numpy math verified
    bit-identical to the reference's jax _log_bucket."""
    e = np.arange(EW)
    d = 1023 - e                      # reversed distance axis
    bkt = _np_log_bucket(d)
    mc = np.clip(bkt + SPAN, 0, 2 * SPAN - 1)
    mp = np.clip(-bkt + SPAN, 0, 2 * SPAN - 1)
    mselk = np.zeros((K2, EW), BF16NP)
    mselk[mc, np.arange(EW)] = 1.0
    mselq = np.zeros((K2, EW), BF16NP)
    mselq[mp, np.arange(EW)] = 1.0
    return mselk, mselq


def _build():
    nc = bacc.Bacc(trn_type="TRN2", num_devices=NCORES, debug=False)

    def din(name, shape, dt=F32):
        return nc.dram_tensor(name, shape, dt, kind="ExternalInput")

    x_d = din("x", [S, D], F32R)
    wq_d, wk_d, wv_d = din("wq", [D, CH], BF16), din("wk", [D, CH], BF16), din("wv", [D, CH], BF16)
    bqs_d, bk_d, bv_d = din("bqs", [CH]), din("bk", [CH]), din("bv", [CH])
    wo_d, boh_d = din("wo", [CH, D], BF16), din("boh", [D])
    rel_d = din("rel", [K2, D])
    lng_d, lnb_d = din("lng", [D]), din("lnb", [D])
    wpk_d, wpq_d = din("wpk", [D, CH], BF16), din("wpq", [D, CH], BF16)
    bpk_d, bpq_d = din("bpk", [CH]), din("bpq", [CH])
    mselk_d, mselq_d = din("mselk", [K2, EW], BF16), din("mselq", [K2, EW], BF16)
    out_d = nc.dram_tensor("out", [S, D], F32, kind="ExternalOutput")

    # internal DRAM band tables (per head)
    c2pt_d = [nc.dram_tensor(f"c2pt{h}", [S, TW], BF16) for h in range(NH)]
    p2ct_d = [nc.dram_tensor(f"p2ct{h}", [S, TW], BF16) for h in range(NH)]

    def bcast_ap(t, n):
        return bass.AP(tensor=t, offset=0, ap=[[0, P], [1, n]])

    def blocks_ap(t, nblk, ncol):
        # fold [nblk*128, ncol] dram tensor into [128, nblk*ncol] sbuf layout
        return bass.AP(tensor=t, offset=0, ap=[[ncol, P], [P * ncol, nblk], [1, ncol]])

    def band_ap(t, blk):
        # band[p, jj] = tbl[128*blk + p, 127 - p + jj]
        return bass.AP(tensor=t, offset=128 * blk * TW + 127, ap=[[TW - 1, P], [1, S]])

    def band_ap4(t, g):
        # 4-block grouped band read: out[p, 1024*b + jj] = tbl[128*(4g+b) + p, 127 - p + jj]
        return bass.AP(tensor=t, offset=128 * (4 * g) * TW + 127,
                       ap=[[TW - 1, P], [128 * TW, 4], [1, S]])

    with tile.TileContext(nc) as tc:
        with (
            tc.tile_pool(name="pers", bufs=1) as pers,
            tc.tile_pool(name="ppT", bufs=2, space="PSUM") as ppT,      # tag tbl: [128,512] f32
            tc.tile_pool(name="ppS", bufs=2, space="PSUM") as ppS,      # tag qk: [128,1024] f32
        ):
            id_f = pers.tile([P, P], F32, tag="idf", name="idf")
            masks.make_identity(nc, id_f[:])
            id_r = pers.tile([P, P], F32R, tag="idr", name="idr")
            nc.vector.tensor_copy(id_r[:], id_f[:])
            id_b = pers.tile([P, P], BF16, tag="idb", name="idb")
            masks.make_identity(nc, id_b[:])

            pkeb = [pers.tile([P, EW], BF16, tag=f"pke{m}", name=f"pke{m}") for m in range(4)]
            pqeb = [pers.tile([P, EW], BF16, tag=f"pqe{m}", name=f"pqe{m}") for m in range(4)]
            qTb = [pers.tile([P, S], BF16, tag=f"qT{m}", name=f"qT{m}") for m in range(4)]
            kTb = [pers.tile([P, S], BF16, tag=f"kT{m}", name=f"kT{m}") for m in range(4)]
            v_aug = [pers.tile([P, 65 * NH], BF16, tag=f"vaug{m}", name=f"vaug{m}") for m in range(8)]
            ctxTn = [pers.tile([P, S], BF16, tag=f"ctxTn{m}", name=f"ctxTn{m}") for m in range(4)]

            # =========== phase 0: prefetch everything, xT+qkv first, then LN chain ===========
            with tc.tile_pool(name="p0", bufs=1) as p0:
                # ---- consolidated input loads (one DMA per tensor) ----
                xA = p0.tile([P, 4 * D], F32R, tag="xsb", name="xA")
                nc.sync.dma_start(xA[:], blocks_ap(x_d, 4, D))
                w_sb = {}
                for nm, t in (("wq", wq_d), ("wk", wk_d), ("wv", wv_d)):
                    w_sb[nm] = p0.tile([P, 8 * CH], BF16, tag=nm, name=nm)
                    nc.sync.dma_start(w_sb[nm][:], blocks_ap(t, 8, CH))
                wpk_sb = p0.tile([P, 8 * CH], BF16, tag="wp", name="wpk_sb")
                nc.scalar.dma_start(wpk_sb[:], blocks_ap(wpk_d, 8, CH))
                mselk_sb = p0.tile([P, 4 * EW], BF16, tag="msel", name="mselk_sb")
                nc.scalar.dma_start(mselk_sb[:], blocks_ap(mselk_d, 4, EW))
                rel_sb = p0.tile([P, 4 * D], F32, tag="relsb", name="relsb")
                nc.scalar.dma_start(rel_sb[:], blocks_ap(rel_d, 4, D))
                lng_sb = p0.tile([P, D], F32, tag="lng", name="lng")
                lnb_sb = p0.tile([P, D], F32, tag="lnb", name="lnb")
                nc.sync.dma_start(lng_sb[:], bcast_ap(lng_d, D))
                nc.sync.dma_start(lnb_sb[:], bcast_ap(lnb_d, D))
                bpk_sb = p0.tile([P, CH], F32, tag="bpk", name="bpk")
                bpq_sb = p0.tile([P, CH], F32, tag="bpq", name="bpq")
                nc.sync.dma_start(bpk_sb[:], bcast_ap(bpk_d, CH))
                nc.sync.dma_start(bpq_sb[:], bcast_ap(bpq_d, CH))
                bv_sb = p0.tile([P, CH], F32, tag="bv", name="bv")
                nc.sync.dma_start(bv_sb[:], bcast_ap(bv_d, CH))
                bias_col = p0.tile([P, 8], F32, tag="biascol", name="biascol")
                for m in range(4):
                    nc.sync.dma_start(bias_col[:, m:m + 1],
                                      bqs_d.ap()[128 * m:128 * (m + 1)].unsqueeze(1))
                    nc.sync.dma_start(bias_col[:, 4 + m:4 + m + 1],
                                      bk_d.ap()[128 * m:128 * (m + 1)].unsqueeze(1))

                # ---- xT transposes + q/k/v projections (PE busy immediately) ----
                # x is staged in two 4-block halves through one rotating slot; the
                # q/k matmuls for s-columns 0:512 only need the first half.
                xT = [p0.tile([P, S], BF16, tag=f"xT{i}", name=f"xT{i}") for i in range(8)]

                def x_transposes(x_half, st0, ppTP2):
                    for st in range(4):
                        for dt_ in range(8):
                            ps = ppTP2.tile([P, P], F32R, tag="tp", name="tp")
                            nc.tensor.matmul(ps[:], x_half[:, D * st + 128 * dt_:D * st + 128 * (dt_ + 1)],
                                             id_r[:], is_transpose=True)
                            if dt_ % 2 == 0:
                                nc.scalar.activation(xT[dt_][:, 128 * (st0 + st):128 * (st0 + st + 1)],
                                                     ps[:].bitcast(F32), AF.Copy)
                            else:
                                nc.vector.tensor_copy(xT[dt_][:, 128 * (st0 + st):128 * (st0 + st + 1)],
                                                      ps[:].bitcast(F32))

                def qk_proj(nc0):
                    for wnm, dst, sc, bc0 in (("wq", qTb, SC_Q, 0), ("wk", kTb, 1.0, 4)):
                        for m in range(4):
                            ps = ppT.tile([P, 512], F32, tag="tbl", name="tbl")
                            for k in range(8):
                                nc.tensor.matmul(ps[:], w_sb[wnm][:, CH * k + 128 * m:CH * k + 128 * (m + 1)],
                                                 xT[k][:, nc0:nc0 + 512], start=(k == 0), stop=(k == 7))
                            nc.scalar.activation(dst[m][:, nc0:nc0 + 512], ps[:], AF.Identity,
                                                 scale=sc, bias=bias_col[:, bc0 + m:bc0 + m + 1])

                with tc.tile_pool(name="ppTP2", bufs=2, space="PSUM") as ppTP2:
                    x_transposes(xA, 0, ppTP2)
                    xB = p0.tile([P, 4 * D], F32R, tag="xsb", name="xB")
                    nc.scalar.dma_start(xB[:], bass.AP(tensor=x_d, offset=4 * P * D,
                                                       ap=[[D, P], [P * D, 4], [1, D]]))
                    qk_proj(0)
                    x_transposes(xB, 4, ppTP2)
                qk_proj(512)
                for m in range(8):
                    ps = ppT.tile([P, 512], F32, tag="tbl", name="tbl")
                    for k in range(8):
                        nc.tensor.matmul(ps[:], xT[k][:, 128 * m:128 * (m + 1)],
                                         w_sb["wv"][:, CH * k:CH * (k + 1)],
                                         start=(k == 0), stop=(k == 7))
                    for h in range(NH):
                        nc.vector.tensor_add(v_aug[m][:, 65 * h:65 * h + 64], ps[:, 64 * h:64 * h + 64],
                                             bv_sb[:, 64 * h:64 * h + 64])
                        nc.gpsimd.memset(v_aug[m][:, 65 * h + 64:65 * h + 65], 1.0)

                # ---- rel_emb LN -> reT -> pos projections -> expansion ----
                st1 = p0.tile([P, 1], F32, tag="lnst1", name="lnst1")
                st2 = p0.tile([P, 1], F32, tag="lnst2", name="lnst2")
                scr = p0.tile([P, D], F32, tag="lnscr", name="lnscr")
                for i in range(4):
                    re_i = rel_sb[:, D * i:D * (i + 1)]
                    nc.vector.tensor_reduce(st1[:], re_i, AX.X, ALU.add, negate=True)
                    nc.vector.tensor_scalar_mul(st1[:], st1[:], 1.0 / D)
                    nc.scalar.activation(re_i, re_i, AF.Identity, bias=st1[:])
                    nc.scalar.activation(scr[:], re_i, AF.Square, accum_out=st2[:])
                    nc.vector.tensor_scalar(st2[:], st2[:], 1.0 / D, 1e-5, ALU.mult, ALU.add)
                    nc.scalar.activation(st2[:], st2[:], AF.Sqrt)
                    nc.vector.reciprocal(st2[:], st2[:])
                    nc.scalar.activation(re_i, re_i, AF.Copy, scale=st2[:])
                    nc.vector.tensor_mul(re_i, re_i, lng_sb[:])
                    nc.vector.tensor_add(re_i, re_i, lnb_sb[:])
                reT = [p0.tile([P, K2], BF16, tag=f"reT{i}", name=f"reT{i}") for i in range(8)]
                with tc.tile_pool(name="ppTP", bufs=2, space="PSUM") as ppTP:
                    for dt_ in range(8):
                        for kt in range(4):
                            ps = ppTP.tile([P, P], F32, tag="tp", name="tp")
                            nc.tensor.matmul(ps[:], rel_sb[:, D * kt + 128 * dt_:D * kt + 128 * (dt_ + 1)],
                                             id_f[:], is_transpose=True)
                            if kt % 2 == 0:
                                nc.scalar.activation(reT[dt_][:, 128 * kt:128 * (kt + 1)], ps[:], AF.Copy)
                            else:
                                nc.vector.tensor_copy(reT[dt_][:, 128 * kt:128 * (kt + 1)], ps[:])

                posk = [p0.tile([P, CH], BF16, tag=f"posk{m}", name=f"posk{m}") for m in range(4)]
                posq = [p0.tile([P, CH], BF16, tag=f"posq{m}", name=f"posq{m}") for m in range(4)]
                wpq_sb = None
                for wp_t, bias_sb, dst in ((wpk_sb, bpk_sb, posk), (None, bpq_sb, posq)):
                    if wp_t is None:
                        wp_t = wpq_sb = p0.tile([P, 8 * CH], BF16, tag="wp", name="wpq_sb")
                        nc.scalar.dma_start(wp_t[:], blocks_ap(wpq_d, 8, CH))
                    for m in range(4):
                        ps = ppT.tile([P, 512], F32, tag="tbl", name="tbl")
                        for k in range(8):
                            nc.tensor.matmul(ps[:], reT[k][:, 128 * m:128 * (m + 1)],
                                             wp_t[:, CH * k:CH * (k + 1)],
                                             start=(k == 0), stop=(k == 7))
                        nc.vector.tensor_add(dst[m][:], ps[:], bias_sb[:])

                for mi, (msel_t, src, dst, sc) in enumerate(
                        ((mselk_sb, posk, pkeb, SC_PK), (None, posq, pqeb, SC_PQ))):
                    if msel_t is None:
                        msel_t = p0.tile([P, 4 * EW], BF16, tag="msel", name="mselq_sb")
                        nc.scalar.dma_start(msel_t[:], blocks_ap(mselq_d, 4, EW))
                    for m in range(4):
                        for nc0 in range(0, EW, 512):
                            ps = ppT.tile([P, 512], F32, tag="tbl", name="tbl")
                            for k in range(4):
                                nc.tensor.matmul(ps[:], src[k][:, 128 * m:128 * (m + 1)],
                                                 msel_t[:, EW * k + nc0:EW * k + nc0 + 512],
                                                 start=(k == 0), stop=(k == 3))
                            if (nc0 // 512) % 2 == 0:
                                nc.scalar.activation(dst[m][:, nc0:nc0 + 512], ps[:], AF.Copy, scale=sc)
                            else:
                                nc.vector.tensor_scalar_mul(dst[m][:, nc0:nc0 + 512], ps[:], sc)

            # =========== phase 1: per-head attention (software-pipelined) ===========
            with (
                tc.tile_pool(name="bands", bufs=4) as bandp,
                tc.tile_pool(name="pbandp", bufs=10) as pbandp,
                tc.tile_pool(name="tblsb", bufs=6) as tblsbp,
                tc.tile_pool(name="probs", bufs=10) as probsp,
                tc.tile_pool(name="small", bufs=2) as smallp,
                tc.tile_pool(name="ppCTX", bufs=1, space="PSUM") as ppCTX,
            ):
                drain_rr = [nc.vector, nc.scalar]

                def emit_tables(h):
                    """c2p + p2c block tables -> DRAM; grouped cband reads."""
                    hi, ho = h // 2, 64 * (h % 2)
                    cband_g = []
                    for tbl_i, (src, dst_d) in enumerate(((pkeb, c2pt_d[h]), (pqeb, p2ct_d[h]))):
                        qk_src = qTb if tbl_i == 0 else kTb
                        for i in range(NB):
                            e0 = 896 - 128 * i
                            tsb = tblsbp.tile([P, TW], BF16, tag="tsb", name="tsb")
                            for ci, (coff, cw) in enumerate(CHUNKS):
                                ps = ppT.tile([P, 512], F32, tag="tbl", name="tbl")
                                nc.tensor.matmul(ps[:, :cw], qk_src[hi][ho:ho + 64, 128 * i:128 * (i + 1)],
                                                 src[hi][ho:ho + 64, e0 + coff:e0 + coff + cw])
                                eng = drain_rr[(i * 3 + ci) % 2]
                                if eng is nc.scalar:
                                    nc.scalar.activation(tsb[:, coff:coff + cw], ps[:, :cw], AF.Copy)
                                else:
                                    eng.tensor_copy(tsb[:, coff:coff + cw], ps[:, :cw])
                            nc.sync.dma_start(dst_d.ap()[128 * i:128 * (i + 1), :], tsb[:])
                            if tbl_i == 0 and i % 4 == 3:
                                bt = bandp.tile([P, 4 * S], F32R, tag="cband", name="cband")
                                nc.gpsimd.dma_start(bt[:].bitcast(F32), band_ap4(c2pt_d[h], i // 4))
                                cband_g.append(bt)
                    return cband_g

                def emit_attn(h, cband_g):
                    hi, ho = h // 2, 64 * (h % 2)
                    pbands = []
                    for jb in range(NB):
                        pband = pbandp.tile([P, S], BF16, tag="pband", name="pband")
                        nc.gpsimd.dma_start(pband[:], band_ap(p2ct_d[h], jb))
                        pbands.append(pband)

                    probs = []
                    for jb in range(NB):
                        sc_ps = ppS.tile([P, S], F32, tag="qk", name="qk")
                        for nc0 in (0, 512):
                            nc.tensor.matmul(sc_ps[:, nc0:nc0 + 512],
                                             kTb[hi][ho:ho + 64, 128 * jb:128 * (jb + 1)],
                                             qTb[hi][ho:ho + 64, nc0:nc0 + 512], start=True, stop=False)
                        for i in range(NB):
                            nc.tensor.matmul(sc_ps[:, 128 * i:128 * (i + 1)].bitcast(F32R),
                                             cband_g[i // 4][:, S * (i % 4) + 128 * jb:S * (i % 4) + 128 * (jb + 1)],
                                             id_r[:], is_transpose=True, start=False, stop=False)
                        for nc0 in (0, 512):
                            nc.tensor.matmul(sc_ps[:, nc0:nc0 + 512], id_b[:],
                                             pbands[jb][:, nc0:nc0 + 512],
                                             start=False, stop=True)
                        pt = probsp.tile([P, S], BF16, tag="probsT", name="probsT")
                        nc.scalar.activation(pt[:], sc_ps[:], AF.Exp)
                        probs.append(pt)

                    cps = ppCTX.tile([P, S], F32, tag="ctx", name="ctx")
                    for jb in range(NB):
                        for nc0 in (0, 512):
                            nc.tensor.matmul(cps[0:65, nc0:nc0 + 512], v_aug[jb][:, 65 * h:65 * h + 65],
                                             probs[jb][:, nc0:nc0 + 512], start=(jb == 0), stop=(jb == 7))
                    rd = smallp.tile([1, S], F32, tag="recip", name="recip")
                    rdb = smallp.tile([64, S], F32, tag="recipb", name="recipb")
                    nc.vector.reciprocal(rd[:], cps[64:65, :])
                    nc.gpsimd.partition_broadcast(rdb[:], rd[:])
                    nc.vector.tensor_mul(ctxTn[hi][ho:ho + 64, :], cps[0:64, :], rdb[:])

                prev = emit_tables(0)
                for h in range(1, NH):
                    cur = emit_tables(h)
                    emit_attn(h - 1, prev)
                    prev = cur
                emit_attn(NH - 1, prev)

            # =========== phase 2: output projection ===========
            with tc.tile_pool(name="p2", bufs=1) as w2, tc.tile_pool(name="p2s", bufs=2) as s2:
                wo_sb = w2.tile([P, 4 * D], BF16, tag="wo", name="wo")
                nc.sync.dma_start(wo_sb[:], blocks_ap(wo_d, 4, D))
                bo_sb = w2.tile([P, D], F32, tag="bo", name="bo")
                nc.sync.dma_start(bo_sb[:], bcast_ap(boh_d, D))
                for m in range(8):
                    ps = ppS.tile([P, S], F32, tag="qk", name="qk")
                    for nc0 in (0, 512):
                        for k in range(4):
                            nc.tensor.matmul(ps[:, nc0:nc0 + 512], ctxTn[k][:, 128 * m:128 * (m + 1)],
                                             wo_sb[:, D * k + nc0:D * k + nc0 + 512],
                                             start=(k == 0), stop=(k == 3))
                    osb = s2.tile([P, D], F32, tag="osb", name="osb")
                    nc.vector.tensor_add(osb[:], ps[:], bo_sb[:])
                    nc.sync.dma_start(out_d.ap()[128 * m:128 * (m + 1), :], osb[:])

    nc.compile()
    return nc


_NC = None


def _prep_in_maps(hidden_states, Wq, bq, Wk, bk, Wv, bv, Wo, bo,
                  rel_emb, ln_g, ln_b, Wpk, bpk, Wpq, bpq):
    mselk, mselq = _msel_matrices()
    f32 = lambda a: np.ascontiguousarray(np.asarray(a, dtype=np.float32))
    b16 = lambda a: np.ascontiguousarray(np.asarray(a, dtype=np.float32).astype(BF16NP))

    in_maps = []
    for core in range(NCORES):
        b, g = core // 2, core % 2
        hs = slice(CH * g, CH * (g + 1))
        in_maps.append({
            "x": f32(hidden_states[b]),
            "wq": b16(np.asarray(Wq)[:, hs]), "wk": b16(np.asarray(Wk)[:, hs]),
            "wv": b16(np.asarray(Wv)[:, hs]),
            "bqs": f32(np.asarray(bq)[hs] * SC_Q), "bk": f32(np.asarray(bk)[hs]),
            "bv": f32(np.asarray(bv)[hs]),
            "wo": b16(np.asarray(Wo)[hs, :]),
            "boh": f32(bo) if g == 0 else np.zeros(D, np.float32),
            "rel": f32(rel_emb), "lng": f32(ln_g), "lnb": f32(ln_b),
            "wpk": b16(np.asarray(Wpk)[:, hs]), "wpq": b16(np.asarray(Wpq)[:, hs]),
            "bpk": f32(np.asarray(bpk)[hs]), "bpq": f32(np.asarray(bpq)[hs]),
            "mselk": mselk, "mselq": mselq,
        })

    return in_maps


def _run(in_maps, trace=False):
    global _NC
    if _NC is None:
        _NC = _build()
    res = run_bass_kernel_spmd(_NC, in_maps, core_ids=list(range(NCORES)), trace=trace)
    out = np.stack([
        res.results[2 * b]["out"].astype(np.float32) + res.results[2 * b + 1]["out"].astype(np.float32)
        for b in range(B)
    ])
    return out, res


def kernel(**inputs):
    out, _ = _run(_prep_in_maps(**inputs))
    return out


def kernel_traced(**inputs):
    in_maps = _prep_in_maps(**inputs)
    try:
        out, res = _run(in_maps, trace=True)
    except ModuleNotFoundError:
        out, res = _run(in_maps, trace=False)
    return out, res
